# revision 4
# baseline (speedup 1.0000x reference)
"""Trainium2 Bass kernel for nn_DenseEdgeEncoder.

Computes, for B=16 graphs of N=256 nodes with 4096 edges each:
    out[b, i, j, :] = edge_attr[e]      if edge e = (i, j) in graph b
                      emb_table[1]      if i == j (self-loop fill)
                      emb_table[2]      otherwise
(the reference's scatter + embedding-lookup formulation reduces to this;
duplicate edges would scatter-add, which the delta-add below preserves).

Strategy (data-parallel over B, 2 graphs per core on 8 cores):
  1. host: route each graph's edges to its core; convert (src, dst) to flat
     row ids of the dense [N*N, D] per-graph image; fold the diagonal fill in
     as N extra tokens; pre-subtract the background vector v2 = emb_table[2]
     from every token payload (scatter is an ADD on top of the background).
  2. device: fill the output with v2 (one broadcast DMA + one DVE widen +
     4x 8MB contiguous HWDGE writes), then dma_scatter_add the token deltas
     (int16 indices -> 32768-row half-spaces, 4 scatters per core).
  3. host: stack per-core outputs to [16, 256, 256, 64].
"""

import os
import numpy as np

import concourse.bacc as bacc
from concourse import mybir
from concourse.bass_utils import run_bass_kernel_spmd
from concourse.bass_interp import get_hw_module
from concourse._compat import cdiv

B = 16
N = 256
D = 64
NCORES = 8
GPC = B // NCORES  # graphs per core = 2
NROWS_G = N * N  # 65536 rows per graph
HALF = NROWS_G // 2  # 32768 (int16-addressable half-space)
NVIEW = GPC * 2  # scatter views per core = 4

LAST_EXEC_NS = None
LAST_RESULTS = None


def _pack_bucket(rows, deltas, cap):
    """Pack one (graph-slot, half) bucket for dma_scatter_add.

    rows: int array in [0, HALF) - target rows in the half-space view.
    deltas: [len(rows), D] f32 payload minus background.
    cap: uniform token capacity (multiple of 128).

    Returns (src [128, (cap/128)*D] f32, idx [128, cap/16] int16).
    Padding tokens add 0.0 to a row unused by real tokens (safe under the
    SDMA read-modify-write with no same-row concurrency).
    """
    n = len(rows)
    assert n <= cap
    cols = cap // 128
    cols16 = cap // 16

    # find an unused row for the zero-delta padding tokens
    used = np.zeros(HALF, bool)
    used[rows] = True
    pad_row = int(np.argmin(used))  # first unused row (HALF >> n always)

    rows_p = np.full(cap, pad_row, np.int64)
    rows_p[:n] = rows
    deltas_p = np.zeros((cap, D), np.float32)
    deltas_p[:n] = deltas

    # src: token i lives at [i % 128, (i // 128)*D : ...]
    src = np.ascontiguousarray(
        deltas_p.reshape(cols, 128, D).transpose(1, 0, 2)
    ).reshape(128, cols * D)
    # idx: token i at [i % 16, i // 16], replicated to all 8 gpsimd groups
    idx2 = rows_p.astype(np.int16).reshape(cols16, 16)
    idx = np.ascontiguousarray(idx2.T)
    idx_rep = np.ascontiguousarray(np.tile(idx, (8, 1)))
    return src, idx_rep


def _route(edge_attr, emb_table, edge_index, batch_vec):
    """Host-side routing: per-core scatter buckets."""
    src, dst = np.asarray(edge_index[0]), np.asarray(edge_index[1])
    batch_vec = np.asarray(batch_vec)
    edge_attr = np.asarray(edge_attr, dtype=np.float32)
    emb_table = np.asarray(emb_table, dtype=np.float32)

    counts = np.bincount(batch_vec, minlength=B)
    starts = np.cumsum(counts) - counts
    g = batch_vec[src]
    ls = src - starts[g]
    ld = dst - starts[g]
    ok = (ls >= 0) & (ls < N) & (ld >= 0) & (ld < N)  # jax drops OOB scatters
    g, ls, ld = g[ok], ls[ok], ld[ok]
    ea = edge_attr[ok]

    v1, v2 = emb_table[1], emb_table[2]
    delta_e = ea - v2[None, :]
    delta_d = (v1 - v2)[None, :].repeat(N, axis=0)
    diag_rows = np.arange(N) * (N + 1)

    # bucket (core, slot, half) -> (rows, deltas)
    buckets = {}
    row = ls * N + ld
    for core in range(NCORES):
        for slot in range(GPC):
            gb = core * GPC + slot
            m = g == gb
            r_all = np.concatenate([row[m], diag_rows])
            d_all = np.concatenate([delta_e[m], delta_d], axis=0)
            for half in range(2):
                hm = (r_all >= half * HALF) & (r_all < (half + 1) * HALF)
                buckets[(core, slot, half)] = (r_all[hm] - half * HALF, d_all[hm])

    cap = max(len(r) for r, _ in buckets.values())
    cap = cdiv(max(cap, 128), 128) * 128
    return buckets, cap, emb_table


def _build_program(cap, reps=1):
    """Build the per-core program. reps>1 repeats the whole body (used only
    for benchmarking: rep r's background waits for rep r-1's scatters)."""
    cols = cap // 128
    cols16 = cap // 16

    nc = bacc.Bacc(
        "TRN2", target_bir_lowering=False, debug=False, num_devices=NCORES
    )
    emb_t = nc.dram_tensor("emb", [3, D], mybir.dt.float32, kind="ExternalInput").ap()
    src_t, idx_t = [], []
    for v in range(NVIEW):
        src_t.append(
            nc.dram_tensor(
                f"src{v}", [128, cols * D], mybir.dt.float32, kind="ExternalInput"
            ).ap()
        )
        idx_t.append(
            nc.dram_tensor(
                f"idx{v}", [128, cols16], mybir.dt.int16, kind="ExternalInput"
            ).ap()
        )
    out_t = nc.dram_tensor(
        "out", [GPC * NROWS_G, D], mybir.dt.float32, kind="ExternalOutput"
    ).ap()
    # four contiguous 8MB chunks: chunk c = rows [c*32768, (c+1)*32768)
    out_chunks = out_t.rearrange("(c p w) d -> c p (w d)", c=NVIEW, p=128, w=256)

    nc.reset()

    with (
        nc.sbuf_tensor([128, N * D], mybir.dt.float32) as bg,
        nc.sbuf_tensor([128, NVIEW * cols * D], mybir.dt.float32) as pay,
        nc.sbuf_tensor([128, NVIEW * cols16], mybir.dt.int16) as idx,
        nc.semaphore() as s_load,
        nc.semaphore() as s_pay,
        nc.semaphore() as s_built,
        nc.semaphore() as s_bg,
        nc.semaphore() as s_scat,
        nc.Block() as block,
    ):

        @block.sync
        def _(sync):
            for r in range(reps):
                if r > 0:
                    # benchmark mode: don't overwrite rows a previous rep's
                    # scatter is still adding to
                    sync.wait_ge(s_scat, 16 * NVIEW * r)
                # v2 row broadcast into all 128 partitions
                sync.dma_start(
                    out=bg[:, 0:D], in_=emb_t[2:3, :].to_broadcast([128, D])
                ).then_inc(s_load, 16)
                # token payloads + indices
                for v in range(NVIEW):
                    sync.dma_start(
                        out=pay[:, v * cols * D : (v + 1) * cols * D],
                        in_=src_t[v][:, :],
                    ).then_inc(s_pay, 16)
                    sync.dma_start(
                        out=idx[:, v * cols16 : (v + 1) * cols16], in_=idx_t[v][:, :]
                    ).then_inc(s_pay, 16)
                # background fill once the image is widened
                sync.wait_ge(s_built, r + 1)
                for c in range(NVIEW):
                    sync.dma_start(out=out_chunks[c], in_=bg[:]).then_inc(s_bg, 16)

        @block.vector
        def _(vector):
            for r in range(reps):
                vector.wait_ge(s_load, 16 * (r + 1))
                bcast = (
                    bg[:, 0:D]
                    .rearrange("p (x d) -> p x d", x=1)
                    .to_broadcast([128, N - 1, D])
                )
                vector.tensor_copy(
                    out=bg[:, D:].rearrange("p (x d) -> p x d", d=D), in_=bcast
                ).then_inc(s_built, 1)

        @block.gpsimd
        def _(gpsimd):
            for r in range(reps):
                gpsimd.wait_ge(s_pay, 16 * 2 * NVIEW * (r + 1))
                for v in range(NVIEW):
                    # view v covers out rows [v*HALF, (v+1)*HALF); its
                    # background is chunk v of this rep
                    gpsimd.wait_ge(s_bg, 16 * (NVIEW * r + v + 1))
                    gpsimd.dma_scatter_add(
                        out_ap=out_t[v * HALF : (v + 1) * HALF, :],
                        in_ap=pay[:, v * cols * D : (v + 1) * cols * D].rearrange(
                            "p (c d) -> p c d", d=D
                        ),
                        idxs_ap=idx[:, v * cols16 : (v + 1) * cols16],
                        num_idxs=cap,
                        num_idxs_reg=cap,
                        elem_size=D,
                    ).then_inc(s_scat, 16)

    nc.compile()
    nc.m = get_hw_module(nc.m)
    return nc


def prepare(edge_attr, emb_table, edge_index, batch_vec):
    """Host routing + program build. Returns (nc, in_maps)."""
    buckets, cap, emb_np = _route(edge_attr, emb_table, edge_index, batch_vec)

    nc = _build_program(cap)

    in_maps = []
    for core in range(NCORES):
        m = {"emb": emb_np}
        for slot in range(GPC):
            for half in range(2):
                v = slot * 2 + half
                rows, deltas = buckets[(core, slot, half)]
                src, idxr = _pack_bucket(rows, deltas, cap)
                m[f"src{v}"] = src
                m[f"idx{v}"] = idxr
        in_maps.append(m)
    return nc, in_maps


def kernel(edge_attr, emb_table, edge_index, batch_vec):
    global LAST_EXEC_NS, LAST_RESULTS
    nc, in_maps = prepare(edge_attr, emb_table, edge_index, batch_vec)

    trace = bool(int(os.environ.get("BASSK_TRACE", "0")))
    res = run_bass_kernel_spmd(nc, in_maps, list(range(NCORES)), trace=trace)
    LAST_EXEC_NS = res.exec_time_ns
    LAST_RESULTS = res

    out = np.empty((B, N, N, D), np.float32)
    for core in range(NCORES):
        blockv = res.results[core]["out"].reshape(GPC, N, N, D)
        out[core * GPC : (core + 1) * GPC] = blockv
    return out


# revision 14
# speedup vs baseline: 21.9957x; 21.9957x over previous
"""Trainium2 Bass kernel for nn_DenseEdgeEncoder.

Computes, for B=16 graphs of N=256 nodes with 4096 edges each:
    out[b, i, j, :] = edge_attr[e]      if edge e = (i, j) in graph b
                      emb_table[1]      if i == j (self-loop fill)
                      emb_table[2]      otherwise
(the reference's scatter + embedding-lookup formulation reduces to this;
duplicate edges would scatter-add, which the delta-add below preserves).

Strategy (data-parallel over B, 2 graphs per core on 8 cores):
  1. host: route each graph's edges to its core; convert (src, dst) to flat
     row ids of the dense [N*N, D] per-graph image; fold the diagonal fill in
     as N extra tokens; pre-subtract the background vector v2 = emb_table[2]
     from every token payload (scatter is an ADD on top of the background).
  2. device: fill the output with v2 (one broadcast DMA + one DVE widen +
     4x 8MB contiguous HWDGE writes), then dma_scatter_add the token deltas
     (int16 indices -> 32768-row half-spaces, 4 scatters per core).
  3. host: stack per-core outputs to [16, 256, 256, 64].
"""

import os
import numpy as np

import concourse.bacc as bacc
from concourse import mybir
from concourse.bass_utils import run_bass_kernel_spmd
from concourse.bass_interp import get_hw_module
from concourse._compat import cdiv

B = 16
N = 256
D = 64
NCORES = 8
GPC = B // NCORES  # graphs per core = 2
NROWS_G = N * N  # 65536 rows per graph
HALF = NROWS_G // 2  # 32768 (int16-addressable half-space)
NVIEW = GPC * 2  # scatter views per core = 4

LAST_EXEC_NS = None
LAST_RESULTS = None


def _pack_bucket(rows, deltas, cap):
    """Pack one (graph-slot, half) bucket for dma_scatter_add.

    rows: int array in [0, HALF) - target rows in the half-space view.
    deltas: [len(rows), D] f32 payload minus background.
    cap: uniform token capacity (multiple of 128).

    Returns (src [128, (cap/128)*D] f32, idx [128, cap/16] int16).
    Padding tokens add 0.0 to a row unused by real tokens (safe under the
    SDMA read-modify-write with no same-row concurrency).
    """
    n = len(rows)
    assert n <= cap
    cols = cap // 128
    cols16 = cap // 16

    # find an unused row for the zero-delta padding tokens
    used = np.zeros(HALF, bool)
    used[rows] = True
    pad_row = int(np.argmin(used))  # first unused row (HALF >> n always)

    rows_p = np.full(cap, pad_row, np.int64)
    rows_p[:n] = rows
    deltas_p = np.zeros((cap, D), np.float32)
    deltas_p[:n] = deltas

    # src: token i lives at [i % 128, (i // 128)*D : ...]
    src = np.ascontiguousarray(
        deltas_p.reshape(cols, 128, D).transpose(1, 0, 2)
    ).reshape(128, cols * D)
    # idx: token i at [i % 16, i // 16], replicated to all 8 gpsimd groups
    idx2 = rows_p.astype(np.int16).reshape(cols16, 16)
    idx = np.ascontiguousarray(idx2.T)
    idx_rep = np.ascontiguousarray(np.tile(idx, (8, 1)))
    return src, idx_rep


def _route(edge_attr, emb_table, edge_index, batch_vec):
    """Host-side routing: per-core scatter buckets."""
    src, dst = np.asarray(edge_index[0]), np.asarray(edge_index[1])
    batch_vec = np.asarray(batch_vec)
    edge_attr = np.asarray(edge_attr, dtype=np.float32)
    emb_table = np.asarray(emb_table, dtype=np.float32)

    counts = np.bincount(batch_vec, minlength=B)
    starts = np.cumsum(counts) - counts
    g = batch_vec[src]
    ls = src - starts[g]
    ld = dst - starts[g]
    ok = (ls >= 0) & (ls < N) & (ld >= 0) & (ld < N)  # jax drops OOB scatters
    g, ls, ld = g[ok], ls[ok], ld[ok]
    ea = edge_attr[ok]

    v1, v2 = emb_table[1], emb_table[2]
    delta_e = ea - v2[None, :]
    delta_d = (v1 - v2)[None, :].repeat(N, axis=0)
    diag_rows = np.arange(N) * (N + 1)

    # bucket (core, slot, half) -> (rows, deltas)
    buckets = {}
    row = ls * N + ld
    for core in range(NCORES):
        for slot in range(GPC):
            gb = core * GPC + slot
            m = g == gb
            r_all = np.concatenate([row[m], diag_rows])
            d_all = np.concatenate([delta_e[m], delta_d], axis=0)
            for half in range(2):
                hm = (r_all >= half * HALF) & (r_all < (half + 1) * HALF)
                buckets[(core, slot, half)] = (r_all[hm] - half * HALF, d_all[hm])

    cap = max(len(r) for r, _ in buckets.values())
    cap = cdiv(max(cap, 128), 128) * 128
    return buckets, cap, emb_table


# background image piece widths in D-blocks: the DVE widen is ~3x slower
# than the write DMAs per byte, so tier the widen (tiny/medium/rest) and
# start writing each tier while the next is still being widened
TIERS = (16, 64, 160, 256)  # cumulative tier ends (exclusive), in D-blocks


def _build_program(cap, reps=1):
    """Build the per-core program. reps>1 repeats the whole body (used only
    for benchmarking: rep r's background waits for rep r-1's scatters).

    Engine split:
      sync (HWDGE queue): v2 broadcast + the 32MB of background writes only
      vector (DVE): widen v2 row image in two pieces so writes start early
      gpsimd (SWDGE): token/idx loads + the 4 scatter-adds

    Semaphore discipline: waits only ever target a semaphore's FULL value at
    that point (per-DMA completions interleave across the 16 SDMA engines, so
    intermediate values can be mixtures of several DMAs).
    """
    cols = cap // 128
    cols16 = cap // 16
    bounds = [0] + [t * D for t in TIERS]  # tier boundaries in elems

    nc = bacc.Bacc(
        "TRN2", target_bir_lowering=False, debug=False, num_devices=NCORES
    )
    emb_t = nc.dram_tensor("emb", [3, D], mybir.dt.float32, kind="ExternalInput").ap()
    src_t = nc.dram_tensor(
        "src", [128, NVIEW * cols * D], mybir.dt.float32, kind="ExternalInput"
    ).ap()
    idx_t = nc.dram_tensor(
        "idx", [128, NVIEW * cols16], mybir.dt.int16, kind="ExternalInput"
    ).ap()
    out_t = nc.dram_tensor(
        "out", [GPC * NROWS_G, D], mybir.dt.float32, kind="ExternalOutput"
    ).ap()
    # four contiguous 8MB chunks: chunk c = out rows [c*HALF, (c+1)*HALF)
    # == exactly scatter view c; split each into left/right column pieces
    out_chunks = out_t.rearrange("(c p w) d -> c p (w d)", c=NVIEW, p=128, w=256)

    nc.reset()

    with (
        nc.sbuf_tensor([128, N * D], mybir.dt.float32) as bg,
        nc.sbuf_tensor([128, NVIEW * cols * D], mybir.dt.float32) as pay,
        nc.sbuf_tensor([128, NVIEW * cols16], mybir.dt.int16) as idx,
        nc.semaphore() as s_load,
        nc.semaphore() as s_pay,
        nc.semaphore() as s_t0,
        nc.semaphore() as s_t1,
        nc.semaphore() as s_t2,
        nc.semaphore() as s_t3,
        nc.semaphore() as s_bgc0,
        nc.semaphore() as s_bgc1,
        nc.semaphore() as s_bgc2,
        nc.semaphore() as s_bgc3,
        nc.semaphore() as s_scat,
        nc.Block() as block,
    ):
        s_bgc = [s_bgc0, s_bgc1, s_bgc2, s_bgc3]
        s_tier = [s_t0, s_t1, s_t2, s_t3]

        @block.sync
        def _(sync):
            for r in range(reps):
                if r > 0:
                    # benchmark mode: previous rep's scatters must finish
                    # before overwriting their rows (and the bg/pay tiles)
                    sync.wait_ge(s_scat, 16 * NVIEW * r)
                # v2 row broadcast into all 128 partitions
                sync.dma_start(
                    out=bg[:, 0:D], in_=emb_t[2:3, :].to_broadcast([128, D])
                ).then_inc(s_load, 16)
                # background writes: each widen tier's pieces start while
                # the next tier is still being widened on DVE
                for t in range(len(TIERS)):
                    lo, hi = bounds[t], bounds[t + 1]
                    sync.wait_ge(s_tier[t], r + 1)
                    for c in range(NVIEW):
                        sync.dma_start(
                            out=out_chunks[c][:, lo:hi], in_=bg[:, lo:hi]
                        ).then_inc(s_bgc[c], 16)

        @block.vector
        def _(vector):
            for r in range(reps):
                vector.wait_ge(s_load, 16 * (r + 1))
                v2blk = bg[:, 0:D].rearrange("p (x d) -> p x d", x=1)
                prev = 1  # first tier's copy starts after the v2 block itself
                for t in range(len(TIERS)):
                    lo, hi = prev * D, bounds[t + 1]
                    vector.tensor_copy(
                        out=bg[:, lo:hi].rearrange("p (x d) -> p x d", d=D),
                        in_=v2blk.to_broadcast([128, TIERS[t] - prev, D]),
                    ).then_inc(s_tier[t], 1)
                    prev = TIERS[t]

        @block.gpsimd
        def _(gpsimd):
            for r in range(reps):
                if r > 0:
                    # pay/idx tiles are read by the previous rep's scatters
                    gpsimd.wait_ge(s_scat, 16 * NVIEW * r)
                gpsimd.dma_start(out=pay[:], in_=src_t[:, :]).then_inc(s_pay, 16)
                gpsimd.dma_start(out=idx[:], in_=idx_t[:, :]).then_inc(s_pay, 16)
                gpsimd.wait_ge(s_pay, 32 * (r + 1))
                for v in range(NVIEW):
                    # view v's background is chunk v (4 tier pieces = 64)
                    gpsimd.wait_ge(s_bgc[v], 16 * len(TIERS) * (r + 1))
                    gpsimd.dma_scatter_add(
                        out_ap=out_t[v * HALF : (v + 1) * HALF, :],
                        in_ap=pay[:, v * cols * D : (v + 1) * cols * D].rearrange(
                            "p (c d) -> p c d", d=D
                        ),
                        idxs_ap=idx[:, v * cols16 : (v + 1) * cols16],
                        num_idxs=cap,
                        num_idxs_reg=cap,
                        elem_size=D,
                    ).then_inc(s_scat, 16)

    nc.compile()
    nc.m = get_hw_module(nc.m)
    return nc


def prepare(edge_attr, emb_table, edge_index, batch_vec):
    """Host routing + program build. Returns (nc, in_maps)."""
    buckets, cap, emb_np = _route(edge_attr, emb_table, edge_index, batch_vec)

    nc = _build_program(cap)

    in_maps = []
    for core in range(NCORES):
        srcs, idxs = [], []
        for slot in range(GPC):
            for half in range(2):
                src, idxr = _pack_bucket(*buckets[(core, slot, half)], cap)
                srcs.append(src)
                idxs.append(idxr)
        in_maps.append(
            {
                "emb": emb_np,
                "src": np.ascontiguousarray(np.concatenate(srcs, axis=1)),
                "idx": np.ascontiguousarray(np.concatenate(idxs, axis=1)),
            }
        )
    return nc, in_maps


def kernel(edge_attr, emb_table, edge_index, batch_vec):
    global LAST_EXEC_NS, LAST_RESULTS
    nc, in_maps = prepare(edge_attr, emb_table, edge_index, batch_vec)

    trace = bool(int(os.environ.get("BASSK_TRACE", "0")))
    res = run_bass_kernel_spmd(nc, in_maps, list(range(NCORES)), trace=trace)
    LAST_EXEC_NS = res.exec_time_ns
    LAST_RESULTS = res

    out = np.empty((B, N, N, D), np.float32)
    for core in range(NCORES):
        blockv = res.results[core]["out"].reshape(GPC, N, N, D)
        out[core * GPC : (core + 1) * GPC] = blockv
    return out
